# revision 1
# baseline (speedup 1.0000x reference)
"""Trainium2 Bass kernel for GQA attention with RoPE (B=2, S=1024, HID=2048,
16 q heads / 4 kv heads, head dim 128, causal).

Sharding: 8 cores = 2 batches x 4 kv-head groups. Core c = b*4 + g handles
batch b and kv head g (query heads 4g..4g+3). Each core computes a partial
output y_part = attn_heads @ wo_shard; the host sums the 4 partials per batch.

Per-core dataflow (matmuls fp32r, moving free dim >= 256):
  Phase A (per 128-row chunk g, software-pipelined 2 deep):
    x chunk --PE transpose--> xT --mm--> q, [k|v] (natural); RoPE on DVE;
    PE transpose q_rope/k_rope -> persistent qT[d,h,s], kT[d,s], v[s,d].
  Phase B/C (per 256-col macro tile, heads pipelined one deep):
    scoresT[sk,sq] = kT_chunk.T @ qT ; expS = exp(scale*s + mask)  (ACT)
    denom_rep = ones.T @ expS ; U^T = v.T-free @ expS   (PE, accumulated)
    rec = exp(-ln(denom))  (ACT) ; uT = U^T * rec  (DVE, fused with copy)
    y = sum_h uT_h.T @ wo_h  (PE) -> SBUF -> DRAM
"""

import sys

import numpy as np

for _p in ("/opt/trn_rl_repo", "/root/.axon_site/_ro/trn_rl_repo"):
    if _p not in sys.path:
        sys.path.append(_p)

from contextlib import ExitStack

import concourse.bass as bass
import concourse.mybir as mybir
from concourse import bacc
from concourse.masks import make_identity
from concourse.tile import TileContext

P = 128           # partitions / head dim / seq chunk
S = 1024          # sequence length
HID = 2048        # model dim
NH = 4            # query heads per core
D = 128           # head dim
TQ = 256          # query macro-tile (matmul moving free dim)
NT = S // TQ      # 4 macro tiles
KC = HID // P     # 16 contraction chunks
NSK = S // P      # 8 key chunks
NG = S // P       # 8 row chunks
F32 = mybir.dt.float32
F32R = mybir.dt.float32r
SCALE = 1.0 / float(np.sqrt(D))
NEG = -30000.0
AL = mybir.AluOpType
AF = mybir.ActivationFunctionType

N_CORES = 8
B = 2
N_KV = 4


def build_nc():
    nc = bacc.Bacc("TRN2", target_bir_lowering=False, debug=False)
    x_d = nc.declare_dram_parameter("x", [S, HID], F32R, isOutput=False)
    cos_d = nc.declare_dram_parameter("cos", [S, D], F32, isOutput=False)
    sin_d = nc.declare_dram_parameter("sin", [S, D], F32, isOutput=False)
    wq_d = nc.declare_dram_parameter("wq", [HID, NH * D], F32R, isOutput=False)
    wk_d = nc.declare_dram_parameter("wk", [HID, D], F32R, isOutput=False)
    wv_d = nc.declare_dram_parameter("wv", [HID, D], F32R, isOutput=False)
    wo_d = nc.declare_dram_parameter("wo", [NH * D, HID], F32R, isOutput=False)
    out_d = nc.declare_dram_parameter("out", [S, HID], F32, isOutput=True)

    with TileContext(nc) as tc, ExitStack() as ctx:
        consts = ctx.enter_context(tc.tile_pool(name="consts", bufs=1))
        wpool = ctx.enter_context(tc.tile_pool(name="wpool", bufs=1))
        persist = ctx.enter_context(tc.tile_pool(name="persist", bufs=1))

        # ---- constants ----
        ident_f32 = consts.tile([P, P], F32, tag="ident_f32")
        make_identity(nc, ident_f32)
        ident = consts.tile([P, P], F32R, tag="ident")
        nc.vector.tensor_copy(ident, ident_f32)
        ones_f32 = consts.tile([P, P], F32, tag="ones_f32")
        nc.vector.memset(ones_f32, 1.0)
        ones = consts.tile([P, P], F32R, tag="ones")
        nc.vector.tensor_copy(ones, ones_f32)

        # ---- weights (partition-chunked layouts), interleaved with x loads ----
        wq_sb = wpool.tile([P, KC, NH * D], F32R, tag="wq")
        wq_r = wq_d[:].rearrange("(c p) n -> p c n", p=P)
        wkv_sb = wpool.tile([P, KC, 2 * D], F32R, tag="wkv")
        wo_sb = wpool.tile([P, NH, HID], F32R, tag="wo")
        wo_r = wo_d[:].rearrange("(h p) n -> p h n", p=P)
        cos_sb = wpool.tile([P, NG, D], F32, tag="cos")
        sin_sb = wpool.tile([P, NG, D], F32, tag="sin")

        # persistent transposed activations
        qT_all = persist.tile([P, NH, S], F32R, tag="qT")   # [d, h, sq]
        kT = persist.tile([P, S], F32R, tag="kT")           # [d, sk]
        vv = persist.tile([P, NSK, D], F32R, tag="vv")      # v natural [sk, d]

        H2 = D // 2

        def rope(dst, src, g, tmp_tag, wk):
            """dst = src*cos + rotate_half(src)*sin, natural layout [P, D]."""
            cos_g = cos_sb[:, g, :]
            sin_g = sin_sb[:, g, :]
            tmp = wk.tile([P, D], F32, tag=tmp_tag)
            nc.vector.scalar_tensor_tensor(
                out=tmp[:, 0:H2], in0=src[:, H2:D], scalar=-1.0,
                in1=sin_g[:, 0:H2], op0=AL.mult, op1=AL.mult,
            )
            nc.vector.tensor_tensor(
                out=tmp[:, H2:D], in0=src[:, 0:H2], in1=sin_g[:, H2:D], op=AL.mult
            )
            nc.vector.tensor_tensor(out=dst, in0=src, in1=cos_g, op=AL.mult)
            nc.vector.tensor_tensor(out=dst, in0=dst, in1=tmp, op=AL.add)

        # ================= fused pipeline =================
        pa = ctx.enter_context(tc.tile_pool(name="pa", bufs=2))
        pb = ctx.enter_context(tc.tile_pool(name="pb", bufs=2))
        ps_mega = ctx.enter_context(tc.tile_pool(name="ps_mega", bufs=6, space="PSUM"))
        ps_qkv = ctx.enter_context(tc.tile_pool(name="ps_qkv", bufs=1, space="PSUM"))

        # dummy matmuls to lift the PE HAM clock gate to 8/8 while the
        # first x/weight DMAs are still in flight
        warm_ps = ps_mega.tile([P, 512], F32, tag="mega", name="warm")
        for _ in range(40):
            nc.tensor.matmul(warm_ps[:, 0:P], ones, ones, start=True, stop=True)
        warm_drain = pa.tile([P, 4], F32, tag="warmdrain", bufs=1)
        nc.vector.tensor_copy(warm_drain, warm_ps[:, 0:4])

        # causal masks for the two diagonal-straddling chunk positions
        m12 = consts.tile([P, 2 * TQ], F32, tag="m12")
        nc.gpsimd.memset(m12, 0.0)
        nc.gpsimd.affine_select(
            out=m12[:, 0:TQ], in_=m12[:, 0:TQ], compare_op=AL.is_ge, fill=NEG,
            base=0, pattern=[[1, TQ]], channel_multiplier=-1,
        )
        nc.gpsimd.affine_select(
            out=m12[:, TQ : 2 * TQ], in_=m12[:, TQ : 2 * TQ],
            compare_op=AL.is_ge, fill=NEG,
            base=-P, pattern=[[1, TQ]], channel_multiplier=-1,
        )

        x_tiles = [None] * NG
        pend = [None] * NG  # g -> (q_ps3, kv_ps, xT)

        def emit_xdma(g):
            x_nat = pa.tile([P, HID], F32R, tag="xnat", bufs=3)
            nc.sync.dma_start(out=x_nat, in_=x_d[g * P : (g + 1) * P, :])
            x_tiles[g] = x_nat

        # DMA order: x0, wq(2), wkv, x1, cos, sin, x2.., wo(4) trailing
        emit_xdma(0)
        nc.sync.dma_start(out=wq_sb[:, 0:4, :], in_=wq_r[:, 0:4, :])
        nc.sync.dma_start(out=wq_sb[:, 4:8, :], in_=wq_r[:, 4:8, :])
        emit_xdma(1)
        nc.sync.dma_start(out=wq_sb[:, 8:12, :], in_=wq_r[:, 8:12, :])
        nc.sync.dma_start(out=wq_sb[:, 12:16, :], in_=wq_r[:, 12:16, :])
        nc.sync.dma_start(
            out=wkv_sb[:, :, 0:D], in_=wk_d[:].rearrange("(c p) n -> p c n", p=P)
        )
        nc.sync.dma_start(
            out=wkv_sb[:, :, D : 2 * D],
            in_=wv_d[:].rearrange("(c p) n -> p c n", p=P),
        )
        emit_xdma(2)
        nc.sync.dma_start(
            out=cos_sb, in_=cos_d[:].rearrange("(c p) d -> p c d", p=P)
        )
        nc.sync.dma_start(
            out=sin_sb, in_=sin_d[:].rearrange("(c p) d -> p c d", p=P)
        )
        wo_next = [0]

        def emit_wo_dma():
            h = wo_next[0]
            if h < NH:
                nc.sync.dma_start(out=wo_sb[:, h, :], in_=wo_r[:, h, :])
                wo_next[0] += 1

        def transposes(g):
            """x chunk -> xT (PE transpose + DVE cast-copy)."""
            x_nat = x_tiles[g]
            xT = pa.tile([P, KC, P], F32R, tag="xT", bufs=2)
            xT_flat = xT.rearrange("p c d -> p (c d)")
            for kb in range(KC // 4):
                tp_ps = ps_mega.tile([P, 4 * P], F32R, tag="mega", name="tp")
                for j in range(4):
                    k = 4 * kb + j
                    nc.tensor.transpose(
                        tp_ps[:, j * P : (j + 1) * P],
                        x_nat[:, k * P : (k + 1) * P],
                        ident,
                    )
                if kb % 2 == 0:
                    nc.vector.tensor_copy(
                        xT_flat[:, kb * 4 * P : (kb + 1) * 4 * P], tp_ps
                    )
                else:
                    nc.scalar.activation(
                        out=xT_flat[:, kb * 4 * P : (kb + 1) * 4 * P], in_=tp_ps,
                        func=AF.Copy,
                    )
            return xT

        def proj(g, xT):
            """q and kv projections for chunk g (PE, accumulating);
            result copied straight out to SBUF to free the PSUM bank."""
            qkv_ps = ps_qkv.tile([P, NH * D + 2 * D], F32, tag="qkv")
            q_ps = qkv_ps[:, 0 : NH * D]
            kv_ps = qkv_ps[:, NH * D : NH * D + 2 * D]
            for k in range(KC):
                nc.tensor.matmul(
                    q_ps, xT[:, k, :], wq_sb[:, k, :],
                    start=(k == 0), stop=(k == KC - 1),
                )
            for k in range(KC):
                nc.tensor.matmul(
                    kv_ps, xT[:, k, :], wkv_sb[:, k, :],
                    start=(k == 0), stop=(k == KC - 1),
                )
            qkv_sb = pa.tile([P, NH * D + 2 * D], F32, tag="qkvsb")
            nc.scalar.activation(out=qkv_sb, in_=qkv_ps, func=AF.Copy)
            return qkv_sb

        def rope_stage(g, qkv_sb):
            """RoPE on q heads + k (DVE), v copy-out."""
            q3 = qkv_sb[:, 0 : NH * D].rearrange("p (h d) -> p h d", h=NH)
            kv_ps = qkv_sb[:, NH * D : NH * D + 2 * D]
            q_rope = pa.tile([P, NH, D], F32R, tag="qrope")
            for h in range(NH):
                rope(q_rope[:, h, :], q3[:, h, :], g, "tmq", pa)
            k_rope = pa.tile([P, D], F32R, tag="krope")
            rope(k_rope, kv_ps[:, 0:D], g, "tmk", pa)
            nc.vector.tensor_copy(vv[:, g, :], kv_ps[:, D : 2 * D])
            return q_rope, k_rope

        def rope_transpose(g, q_rope, k_rope):
            """Transpose RoPE'd q/k into persistent qT_all / kT."""
            tq_ps = ps_mega.tile([P, 4 * P], F32R, tag="mega", name="tq")
            for h in range(NH):
                nc.tensor.transpose(
                    tq_ps[:, h * P : (h + 1) * P], q_rope[:, h, :], ident
                )
            nc.vector.tensor_copy(
                qT_all[:, :, g * P : (g + 1) * P],
                tq_ps.rearrange("p (h d) -> p h d", h=NH),
            )
            tk_ps = ps_mega.tile([P, 4 * P], F32R, tag="mega", name="tk")
            nc.tensor.transpose(tk_ps[:, 0:P], k_rope, ident)
            nc.vector.tensor_copy(kT[:, g * P : (g + 1) * P], tk_ps[:, 0:P])

        # 2-deep software pipeline over chunks
        ropes = [None] * NG
        attn_todo = []  # deferred attention head-steps, emitted between A work

        def emit_phase_a(g):
            if g >= 2:
                gg = g - 2
                sc = nc.named_scope(f"rope_{gg}"); sc.__enter__()
                ropes[gg] = rope_stage(gg, pend[gg][1])
                sc.__exit__(None, None, None)
            if g < NG:
                if g + 3 < NG:
                    emit_xdma(g + 3)
                if g >= 3:
                    emit_wo_dma()
                    emit_wo_dma()
                sc = nc.named_scope(f"tp_{g}"); sc.__enter__()
                xT = transposes(g)
                sc.__exit__(None, None, None)
                pend[g] = [xT, None, None]
            if g >= 1 and g - 1 < NG:
                gg = g - 1
                sc = nc.named_scope(f"proj_{gg}"); sc.__enter__()
                qkv_sb = proj(gg, pend[gg][0])
                sc.__exit__(None, None, None)
                pend[gg][1] = qkv_sb
            if g >= 2:
                gg = g - 2
                sc = nc.named_scope(f"ropeT_{gg}"); sc.__enter__()
                rope_transpose(gg, *ropes[gg])
                sc.__exit__(None, None, None)
                pend[gg] = None


        def scores_head(t, h):
            """scoresT + exp for head h of macro tile t -> expst tile.

            Chunk pairs share one full PSUM bank so the causal mask is a
            single DVE add and exp is one ACT op per pair."""
            qT_h = qT_all[:, h, t * TQ : (t + 1) * TQ]
            expst = pb.tile([P, NSK, TQ], F32R, tag="expst", bufs=3)
            expst_flat = expst.rearrange("p c f -> p (c f)")
            for pi in range(t + 1):
                s_ps = ps_mega.tile([P, 2 * TQ], F32, tag="mega", name="s")
                for half in range(2):
                    ik = 2 * pi + half
                    nc.tensor.matmul(
                        s_ps[:, half * TQ : (half + 1) * TQ],
                        kT[:, ik * P : (ik + 1) * P], qT_h,
                        start=True, stop=True,
                    )
                if pi == t:
                    nc.vector.tensor_tensor(out=s_ps, in0=s_ps, in1=m12, op=AL.add)
                nc.scalar.activation(
                    out=expst_flat[:, pi * 2 * TQ : (pi + 1) * 2 * TQ],
                    in_=s_ps, func=AF.Exp, scale=SCALE,
                )
            return expst

        def dnpv_head(t, h, expst, uT_t):
            """denominator + PV matmuls, then normalize into uT_t (DVE)."""
            nsk = 2 * (t + 1)
            u_ps = ps_mega.tile([P, 2 * TQ], F32, tag="mega", name="u")[:, 0:TQ]
            den_ps = ps_mega.tile([P, 2 * TQ], F32, tag="mega", name="den")[:, 0:TQ]
            for ik in range(nsk):
                nc.tensor.matmul(
                    den_ps, ones, expst[:, ik, :],
                    start=(ik == 0), stop=(ik == nsk - 1),
                )
            rec = pb.tile([P, TQ], F32, tag="rec", bufs=2)
            nc.vector.reciprocal(rec, den_ps)
            for ik in range(nsk):
                nc.tensor.matmul(
                    u_ps, vv[:, ik, :], expst[:, ik, :],
                    start=(ik == 0), stop=(ik == nsk - 1),
                )
            nc.vector.tensor_tensor(
                out=uT_t[:, h, :], in0=u_ps, in1=rec, op=AL.mult
            )

        def wo_stage(t, uT_t):
            for sub in range(2):
                g = 2 * t + sub
                for n in range(HID // 512):
                    y_ps = ps_mega.tile([P, 512], F32, tag="mega", name="y")
                    for h in range(NH):
                        nc.tensor.matmul(
                            y_ps,
                            uT_t[:, h, sub * P : (sub + 1) * P],
                            wo_sb[:, h, n * 512 : (n + 1) * 512],
                            start=(h == 0), stop=(h == NH - 1),
                        )
                    y_sb = pb.tile([P, 512], F32, tag="ysb", bufs=2)
                    nc.vector.tensor_copy(y_sb, y_ps)
                    nc.gpsimd.dma_start(
                        out=out_d[g * P : (g + 1) * P, n * 512 : (n + 1) * 512],
                        in_=y_sb,
                    )

        # attention head-steps, pipelined one deep with wo lagging two
        # steps; consumed interleaved with phase-A iterations
        steps = [(t, h) for t in range(NT) for h in range(NH)]
        uts = {}
        att_i = [0]

        def emit_attention_step():
            i = att_i[0]
            if i >= len(steps) + 2:
                return False
            if i < len(steps):
                t, h = steps[i]
                if h == 0:
                    uts[t] = pb.tile([P, NH, TQ], F32R, tag="uT", name=f"uT{t}")
                sc = nc.named_scope(f"sc_{t}_{h}"); sc.__enter__()
                uts[(t, h)] = scores_head(t, h)
                sc.__exit__(None, None, None)
            if 1 <= i < len(steps) + 1:
                t, h = steps[i - 1]
                sc = nc.named_scope(f"dnpv_{t}_{h}"); sc.__enter__()
                dnpv_head(t, h, uts.pop((t, h)), uts[t])
                sc.__exit__(None, None, None)
            if i >= 2 and (i - 2) % NH == NH - 1:
                t = steps[i - 2][0]
                sc = nc.named_scope(f"wo_{t}"); sc.__enter__()
                wo_stage(t, uts.pop(t))
                sc.__exit__(None, None, None)
            att_i[0] += 1
            return True

        # drive: phase-A iteration g, then any attention steps whose
        # inputs (kT/v/qT up to chunk 2t+1) are complete after ropeT_{g-2}
        for g in range(NG + 2):
            emit_phase_a(g)
            done_g = g - 2  # ropeT for this chunk just emitted
            while att_i[0] < len(steps) + 2:
                i = att_i[0]
                if i < len(steps):
                    t, _h = steps[i]
                    if 2 * t + 1 > done_g:
                        break
                emit_attention_step()
        emit_wo_dma()
        emit_wo_dma()
        emit_wo_dma()
        emit_wo_dma()
        while emit_attention_step():
            pass

    nc.compile()
    return nc


def shard_inputs(x, cos, sin, wq, wk, wv, wo):
    """Build per-core input maps: core = b*4 + g."""
    in_maps = []
    for c in range(N_CORES):
        b, g = divmod(c, N_KV)
        in_maps.append(
            {
                "x": np.ascontiguousarray(x[b]),
                "cos": np.ascontiguousarray(cos),
                "sin": np.ascontiguousarray(sin),
                "wq": np.ascontiguousarray(wq[:, g * NH * D : (g + 1) * NH * D]),
                "wk": np.ascontiguousarray(wk[:, g * D : (g + 1) * D]),
                "wv": np.ascontiguousarray(wv[:, g * D : (g + 1) * D]),
                "wo": np.ascontiguousarray(wo[g * NH * D : (g + 1) * NH * D, :]),
            }
        )
    return in_maps


_NC_CACHE = {}


def get_nc():
    if "nc" not in _NC_CACHE:
        _NC_CACHE["nc"] = build_nc()
    return _NC_CACHE["nc"]


def kernel(x, cos, sin, wq, wk, wv, wo, _trace=False):
    from concourse.bass_utils import run_bass_kernel_spmd

    x = np.asarray(x, dtype=np.float32)
    cos = np.asarray(cos, dtype=np.float32)
    sin = np.asarray(sin, dtype=np.float32)
    wq = np.asarray(wq, dtype=np.float32)
    wk = np.asarray(wk, dtype=np.float32)
    wv = np.asarray(wv, dtype=np.float32)
    wo = np.asarray(wo, dtype=np.float32)

    nc = get_nc()
    in_maps = shard_inputs(x, cos, sin, wq, wk, wv, wo)
    res = run_bass_kernel_spmd(nc, in_maps, list(range(N_CORES)), trace=_trace)
    parts = [np.asarray(res.results[c]["out"], dtype=np.float32) for c in range(N_CORES)]
    y = np.stack(
        [sum(parts[b * N_KV + g] for g in range(N_KV)) for b in range(B)], axis=0
    )
    if _trace:
        kernel.last_result = res
    return y



# revision 11
# speedup vs baseline: 1.1845x; 1.1845x over previous
"""Trainium2 Bass kernel for GQA attention with RoPE (B=2, S=1024, HID=2048,
16 q heads / 4 kv heads, head dim 128, causal).

Sharding: 8 cores = 2 batches x 4 kv-head groups. Core c = b*4 + g handles
batch b and kv head g (query heads 4g..4g+3). Each core computes a partial
output y_part = attn_heads @ wo_shard; the host sums the 4 partials per batch.

All tensors fp16 on the wire and in SBUF (host casts inputs; host upcasts and
sums the fp16 partials). Matmuls fp16 (1 cyc/row), except the softmax
denominator which runs as fp8e4m3 DoubleRow over chunk pairs (expst8 is a DVE
cast of the fp16 expst). Causal masking is multiplicative-zero on expst via
gpsimd affine_select (Pool engine), so the DVE stays out of the mask path.

Per-core dataflow:
  Phase A (per 128-row chunk g, software-pipelined):
    x chunk --PE transpose--> xT --mm--> q,k,v (natural); batched RoPE on DVE
    (broadcast cos/sin over the 5 q/k blocks); PE transpose q_rope/k_rope ->
    persistent qT[d,h,s], kT[d,s]; v natural -> vv[s,d].
  Attention (per 256-col tile t, head h, 2-stage pipelined; light doses
  interleave into phase A, the bulk runs after):
    scoresT[sk,sq] = kT_chunk.T @ qT ; exp on ACT -> expst f16 ; diagonal
    causal zeroing on Pool ; den = DoubleRow fp8 ones.T @ expst8 ; U^T
    accumulated fp16 ; rec = 1/den (DVE) ; uT = U^T * rec (DVE, f16).
    wo: y[g,:] = sum_h uT_h.T @ wo_h -> y_sb f16 -> DRAM (one DMA per row).
"""

import sys

import numpy as np

for _p in ("/opt/trn_rl_repo", "/root/.axon_site/_ro/trn_rl_repo"):
    if _p not in sys.path:
        sys.path.append(_p)

from contextlib import ExitStack

import concourse.bass as bass
import concourse.mybir as mybir
from concourse import bacc
from concourse.masks import make_identity
from concourse.tile import TileContext

P = 128           # partitions / head dim / seq chunk
S = 1024          # sequence length
HID = 2048        # model dim
NH = 4            # query heads per core
D = 128           # head dim
TQ = 256          # query macro-tile
NT = S // TQ      # 4 macro tiles
KC = HID // P     # 16 contraction chunks
NSK = S // P      # 8 key chunks
NG = S // P       # 8 row chunks
H2 = D // 2
F32 = mybir.dt.float32
F16 = mybir.dt.float16
F8 = mybir.dt.float8e4
SCALE = 1.0 / float(np.sqrt(D))
AL = mybir.AluOpType
AF = mybir.ActivationFunctionType
DR = mybir.MatmulPerfMode.DoubleRow

USE_F8_DEN = False

N_CORES = 8
B = 2
N_KV = 4


def build_nc():
    nc = bacc.Bacc("TRN2", target_bir_lowering=False, debug=False)
    x_d = nc.declare_dram_parameter("x", [S, HID], F16, isOutput=False)
    cos_d = nc.declare_dram_parameter("cos", [S, D], F16, isOutput=False)
    sin_d = nc.declare_dram_parameter("sin", [S, D], F16, isOutput=False)
    wq_d = nc.declare_dram_parameter("wq", [HID, NH * D], F16, isOutput=False)
    wk_d = nc.declare_dram_parameter("wk", [HID, D], F16, isOutput=False)
    wv_d = nc.declare_dram_parameter("wv", [HID, D], F16, isOutput=False)
    wo_d = nc.declare_dram_parameter("wo", [NH * D, HID], F16, isOutput=False)
    out_d = nc.declare_dram_parameter("out", [S, HID], F16, isOutput=True)

    with TileContext(nc) as tc, ExitStack() as ctx:
        consts = ctx.enter_context(tc.tile_pool(name="consts", bufs=1))
        wpool = ctx.enter_context(tc.tile_pool(name="wpool", bufs=1))
        persist = ctx.enter_context(tc.tile_pool(name="persist", bufs=1))

        # ---- tile declarations (DMAs can start before consts are built) ----
        ident_f32 = consts.tile([P, P], F32, tag="ident_f32")
        ident = consts.tile([P, P], F16, tag="ident")
        ones8 = consts.tile([P, 2, P], F8, tag="ones8")
        ones16 = consts.tile([P, P], F16, tag="ones16")

        # ---- weights (partition-chunked layouts) ----
        wq_sb = wpool.tile([P, KC, NH * D], F16, tag="wq")
        wq_r = wq_d[:].rearrange("(c p) n -> p c n", p=P)
        wkv_sb = wpool.tile([P, KC, 2 * D], F16, tag="wkv")
        wo_sb = wpool.tile([P, NH, HID], F16, tag="wo")
        wo_r = wo_d[:].rearrange("(h p) n -> p h n", p=P)
        cos_sb = wpool.tile([P, NG, D], F16, tag="cos")
        sin_sb = wpool.tile([P, NG, D], F16, tag="sin")

        # persistent transposed activations
        qT_all = persist.tile([P, NH, S], F16, tag="qT")   # [d, h, sq]
        kT = persist.tile([P, S], F16, tag="kT")           # [d, sk]
        vv = persist.tile([P, NSK, D], F16, tag="vv")      # v natural [sk, d]

        # ---- SBUF working pools ----
        pa = ctx.enter_context(tc.tile_pool(name="pa", bufs=2))
        pb = ctx.enter_context(tc.tile_pool(name="pb", bufs=2))
        # ---- PSUM (8 banks): qkv 2, tp 2, s 2, ud 2; y shares the s ring ----
        ps_qkv = ctx.enter_context(tc.tile_pool(name="ps_qkv", bufs=1, space="PSUM"))
        ps_tp = ctx.enter_context(tc.tile_pool(name="ps_tp", bufs=2, space="PSUM"))
        ps_s = ctx.enter_context(tc.tile_pool(name="ps_s", bufs=2, space="PSUM"))
        ps_ud = ctx.enter_context(tc.tile_pool(name="ps_ud", bufs=2, space="PSUM"))

        x_tiles = [None] * NG

        def emit_xdma(g):
            x_nat = pa.tile([P, HID], F16, tag="xnat", bufs=4)
            nc.sync.dma_start(out=x_nat, in_=x_d[g * P : (g + 1) * P, :])
            x_tiles[g] = x_nat

        # DMA order: x0 + first wq chunk first, then interleave
        emit_xdma(0)
        nc.sync.dma_start(out=wq_sb[:, 0:4, :], in_=wq_r[:, 0:4, :])
        emit_xdma(1)
        nc.sync.dma_start(out=wq_sb[:, 4:8, :], in_=wq_r[:, 4:8, :])

        # ---- constants (gpsimd/DVE work overlapping the DMAs) ----
        make_identity(nc, ident_f32)
        nc.vector.tensor_copy(ident, ident_f32)
        nc.vector.memset(ones8, 1.0)
        nc.vector.memset(ones16, 1.0)

        # warm up the PE clock while the first DMAs are in flight
        warm_ps = ps_s.tile([P, 2 * TQ], F32, tag="s", name="warm")
        for _ in range(40):
            nc.tensor.matmul(warm_ps[:, 0:P], ident, ident, start=True, stop=True)
        warm_drain = pa.tile([P, 4], F32, tag="warmdrain", bufs=1)
        nc.vector.tensor_copy(warm_drain, warm_ps[:, 0:4])

        emit_xdma(2)
        nc.sync.dma_start(out=wq_sb[:, 8:12, :], in_=wq_r[:, 8:12, :])
        nc.sync.dma_start(out=wq_sb[:, 12:16, :], in_=wq_r[:, 12:16, :])
        emit_xdma(3)
        nc.sync.dma_start(
            out=wkv_sb[:, :, 0:D], in_=wk_d[:].rearrange("(c p) n -> p c n", p=P)
        )
        nc.sync.dma_start(
            out=wkv_sb[:, :, D : 2 * D],
            in_=wv_d[:].rearrange("(c p) n -> p c n", p=P),
        )
        nc.sync.dma_start(out=cos_sb, in_=cos_d[:].rearrange("(c p) d -> p c d", p=P))
        nc.sync.dma_start(out=sin_sb, in_=sin_d[:].rearrange("(c p) d -> p c d", p=P))
        wo_next = [0]

        def emit_wo_dma():
            h = wo_next[0]
            if h < NH:
                nc.sync.dma_start(out=wo_sb[:, h, :], in_=wo_r[:, h, :])
                wo_next[0] += 1

        def transposes(g):
            """x chunk -> xT (PE transpose, f16)."""
            x_nat = x_tiles[g]
            xT = pa.tile([P, KC, P], F16, tag="xT", bufs=2)
            xT_flat = xT.rearrange("p c d -> p (c d)")
            for kb in range(KC // 4):
                tp_ps = ps_tp.tile([P, 5 * P], F16, tag="tp", name="tp")
                for j in range(4):
                    k = 4 * kb + j
                    nc.tensor.transpose(
                        tp_ps[:, j * P : (j + 1) * P],
                        x_nat[:, k * P : (k + 1) * P],
                        ident,
                    )
                if kb % 2 == 0:
                    nc.vector.tensor_copy(
                        xT_flat[:, kb * 4 * P : (kb + 1) * 4 * P], tp_ps[:, 0 : 4 * P]
                    )
                else:
                    nc.scalar.activation(
                        out=xT_flat[:, kb * 4 * P : (kb + 1) * 4 * P],
                        in_=tp_ps[:, 0 : 4 * P],
                        func=AF.Copy,
                    )
            return xT

        def proj(g, xT):
            """q, k, v projections for chunk g (PE, accumulating in PSUM)."""
            qkv_ps = ps_qkv.tile([P, NH * D + 2 * D], F32, tag="qkv")
            q_ps = qkv_ps[:, 0 : NH * D]
            kv_ps = qkv_ps[:, NH * D : NH * D + 2 * D]
            for k in range(KC):
                nc.tensor.matmul(
                    q_ps, xT[:, k, :], wq_sb[:, k, :],
                    start=(k == 0), stop=(k == KC - 1),
                )
            for k in range(KC):
                nc.tensor.matmul(
                    kv_ps, xT[:, k, :], wkv_sb[:, k, :],
                    start=(k == 0), stop=(k == KC - 1),
                )
            # copy-out split across ACT (q) and DVE (kv) to free the bank fast
            qkv_sb = pa.tile([P, NH * D + 2 * D], F16, tag="qkvsb")
            nc.scalar.activation(
                out=qkv_sb[:, 0 : NH * D], in_=q_ps, func=AF.Copy
            )
            nc.vector.tensor_copy(qkv_sb[:, NH * D :], kv_ps)
            return qkv_sb

        def rope_stage(g, qkv_sb):
            """Batched RoPE over the 5 q/k blocks (DVE, broadcast cos/sin)."""
            qk = qkv_sb[:, 0 : 5 * D].rearrange("p (f d) -> p f d", d=D)
            sin_lo = sin_sb[:, g : g + 1, 0:H2].to_broadcast((P, 5, H2))
            sin_hi = sin_sb[:, g : g + 1, H2:D].to_broadcast((P, 5, H2))
            cos_bc = cos_sb[:, g : g + 1, :].to_broadcast((P, 5, D))
            tmp = pa.tile([P, 5, D], F16, tag="ropetmp")
            dst = pa.tile([P, 5, D], F16, tag="qkrope")
            nc.vector.scalar_tensor_tensor(
                out=tmp[:, :, 0:H2], in0=qk[:, :, H2:D], scalar=-1.0,
                in1=sin_lo, op0=AL.mult, op1=AL.mult,
            )
            nc.vector.tensor_tensor(
                out=tmp[:, :, H2:D], in0=qk[:, :, 0:H2], in1=sin_hi, op=AL.mult
            )
            nc.vector.tensor_tensor(out=dst, in0=qk, in1=cos_bc, op=AL.mult)
            nc.vector.tensor_tensor(
                out=dst.rearrange("p f d -> p (f d)"),
                in0=dst.rearrange("p f d -> p (f d)"),
                in1=tmp.rearrange("p f d -> p (f d)"),
                op=AL.add,
            )
            # v copy-out (cast f16)
            nc.vector.tensor_copy(vv[:, g, :], qkv_sb[:, 5 * D : 6 * D])
            return dst

        def rope_transpose(g, dst):
            """Transpose RoPE'd q/k into persistent qT_all / kT."""
            tq_ps = ps_tp.tile([P, 5 * P], F16, tag="tp", name="tq")
            for f in range(5):
                nc.tensor.transpose(
                    tq_ps[:, f * P : (f + 1) * P], dst[:, f, :], ident
                )
            nc.vector.tensor_copy(
                qT_all[:, :, g * P : (g + 1) * P],
                tq_ps[:, 0 : 4 * P].rearrange("p (h d) -> p h d", h=NH),
            )
            nc.scalar.activation(
                out=kT[:, g * P : (g + 1) * P], in_=tq_ps[:, 4 * P : 5 * P],
                func=AF.Copy,
            )

        ropes = [None] * NG
        pend = [None] * NG

        def emit_phase_a(g):
            if g >= 2:
                gg = g - 2
                with nc.named_scope(f"rope_{gg}"):
                    ropes[gg] = rope_stage(gg, pend[gg][1])
            if g < NG:
                if g + 4 < NG:
                    emit_xdma(g + 4)
                if g in (2, 3, 4, 5):
                    emit_wo_dma()
                with nc.named_scope(f"tp_{g}"):
                    xT = transposes(g)
                pend[g] = [xT, None]
            if g >= 1 and g - 1 < NG:
                gg = g - 1
                with nc.named_scope(f"proj_{gg}"):
                    qkv_sb = proj(gg, pend[gg][0])
                pend[gg][1] = qkv_sb
            if g >= 2:
                gg = g - 2
                with nc.named_scope(f"ropeT_{gg}"):
                    rope_transpose(gg, ropes[gg])
                pend[gg] = None

        # ---------- attention ----------
        def scores_head(t, h):
            """scoresT + exp + causal zeroing -> (expst f16, expst8 f8)."""
            qT_h = qT_all[:, h, t * TQ : (t + 1) * TQ]
            expst = pb.tile([P, NSK, TQ], F16, tag="expst", bufs=3)
            expst_flat = expst.rearrange("p c f -> p (c f)")
            if USE_F8_DEN:
                expst8 = pb.tile([P, NSK, TQ], F8, tag="expst8", bufs=3)
            else:
                expst8 = None
            for pi in range(t + 1):
                s_ps = ps_s.tile([P, 2 * TQ], F32, tag="s", name="s")
                for half in range(2):
                    ik = 2 * pi + half
                    nc.tensor.matmul(
                        s_ps[:, half * TQ : (half + 1) * TQ],
                        kT[:, ik * P : (ik + 1) * P], qT_h,
                        start=True, stop=True,
                    )
                nc.scalar.activation(
                    out=expst_flat[:, pi * 2 * TQ : (pi + 1) * 2 * TQ],
                    in_=s_ps, func=AF.Exp, scale=SCALE,
                )
                if pi == t:
                    # causal zeroing of the two diagonal chunks (Pool engine)
                    nc.gpsimd.affine_select(
                        out=expst[:, 2 * t, :], in_=expst[:, 2 * t, :],
                        compare_op=AL.is_ge, fill=0.0,
                        base=0, pattern=[[1, TQ]], channel_multiplier=-1,
                    )
                    nc.gpsimd.affine_select(
                        out=expst[:, 2 * t + 1, :], in_=expst[:, 2 * t + 1, :],
                        compare_op=AL.is_ge, fill=0.0,
                        base=-P, pattern=[[1, TQ]], channel_multiplier=-1,
                    )
                if USE_F8_DEN:
                    # scale by 1+2^-5 so the cast's truncation toward zero
                    # is centered (e4m3 step = 2^-3 relative); den then
                    # averages out instead of sitting ~3% low
                    nc.vector.tensor_scalar_mul(
                        out=expst8.rearrange("p c f -> p (c f)")[
                            :, pi * 2 * TQ : (pi + 1) * 2 * TQ
                        ],
                        in0=expst_flat[:, pi * 2 * TQ : (pi + 1) * 2 * TQ],
                        scalar1=1.03125,
                    )
            return expst, expst8

        def dnpv_head(t, h, exps, uT_t):
            """denominator + PV matmuls, then normalize into uT_t (DVE)."""
            expst, expst8 = exps
            nsk = 2 * (t + 1)
            ud_ps = ps_ud.tile([P, 2 * TQ], F32, tag="ud", name="ud")
            u_ps = ud_ps[:, 0:TQ]
            den_ps = ud_ps[:, TQ : 2 * TQ]
            if USE_F8_DEN:
                for pi in range(t + 1):
                    nc.tensor.matmul(
                        den_ps, ones8,
                        expst8[:, 2 * pi : 2 * pi + 2, :],
                        start=(pi == 0), stop=(pi == t), perf_mode=DR,
                    )
            else:
                for ik in range(nsk):
                    nc.tensor.matmul(
                        den_ps, ones16, expst[:, ik, :],
                        start=(ik == 0), stop=(ik == nsk - 1),
                    )
            rec = pb.tile([P, TQ], F32, tag="rec", bufs=2)
            nc.vector.reciprocal(rec, den_ps)
            for ik in range(nsk):
                nc.tensor.matmul(
                    u_ps, vv[:, ik, :], expst[:, ik, :],
                    start=(ik == 0), stop=(ik == nsk - 1),
                )
            nc.vector.tensor_tensor(
                out=uT_t[:, h, :], in0=u_ps, in1=rec, op=AL.mult
            )

        def wo_stage(t, uT_t):
            for sub in range(2):
                g = 2 * t + sub
                y_sb = pb.tile([P, HID], F16, tag="ysb", bufs=2)
                for n in range(HID // 512):
                    y_ps = ps_s.tile([P, 512], F32, tag="s", name="y")
                    for h in range(NH):
                        nc.tensor.matmul(
                            y_ps,
                            uT_t[:, h, sub * P : (sub + 1) * P],
                            wo_sb[:, h, n * 512 : (n + 1) * 512],
                            start=(h == 0), stop=(h == NH - 1),
                        )
                    if n % 2 == 0:
                        nc.vector.tensor_copy(
                            y_sb[:, n * 512 : (n + 1) * 512], y_ps
                        )
                    else:
                        nc.scalar.activation(
                            out=y_sb[:, n * 512 : (n + 1) * 512], in_=y_ps,
                            func=AF.Copy,
                        )
                nc.gpsimd.dma_start(
                    out=out_d[g * P : (g + 1) * P, :], in_=y_sb
                )

        steps = [(t, h) for t in range(NT) for h in range(NH)]
        uts = {}
        att_i = [0]

        def emit_attention_step():
            i = att_i[0]
            if i >= len(steps) + 2:
                return False
            if i < len(steps):
                t, h = steps[i]
                if h == 0:
                    uts[t] = pb.tile([P, NH, TQ], F16, tag="uT", name=f"uT{t}")
                with nc.named_scope(f"sc_{t}_{h}"):
                    uts[(t, h)] = scores_head(t, h)
            if 1 <= i < len(steps) + 1:
                t, h = steps[i - 1]
                with nc.named_scope(f"dnpv_{t}_{h}"):
                    dnpv_head(t, h, uts.pop((t, h)), uts[t])
            if i >= 2 and (i - 2) % NH == NH - 1:
                t = steps[i - 2][0]
                with nc.named_scope(f"wo_{t}"):
                    wo_stage(t, uts.pop(t))
            att_i[0] += 1
            return True

        def att_ready():
            i = att_i[0]
            if i >= len(steps) + 2:
                return False
            if i < len(steps):
                t, _h = steps[i]
                if 2 * t + 1 > done_g[0]:
                    return False
            return True

        # drive: phase A strictly prioritized; 1 attention step per
        # iteration to fill PE bubbles, the bulk after phase A completes
        done_g = [-1]
        for g in range(NG + 2):
            emit_phase_a(g)
            done_g[0] = g - 2
            if g >= 3 and att_ready():
                emit_attention_step()
        while emit_attention_step():
            pass

    nc.compile()
    return nc


def shard_inputs(x, cos, sin, wq, wk, wv, wo):
    """Build per-core input maps (fp16): core = b*4 + g."""
    f16 = np.float16
    in_maps = []
    for c in range(N_CORES):
        b, g = divmod(c, N_KV)
        in_maps.append(
            {
                "x": np.ascontiguousarray(x[b], dtype=f16),
                "cos": np.ascontiguousarray(cos, dtype=f16),
                "sin": np.ascontiguousarray(sin, dtype=f16),
                "wq": np.ascontiguousarray(
                    wq[:, g * NH * D : (g + 1) * NH * D], dtype=f16
                ),
                "wk": np.ascontiguousarray(wk[:, g * D : (g + 1) * D], dtype=f16),
                "wv": np.ascontiguousarray(wv[:, g * D : (g + 1) * D], dtype=f16),
                "wo": np.ascontiguousarray(
                    wo[g * NH * D : (g + 1) * NH * D, :], dtype=f16
                ),
            }
        )
    return in_maps


_NC_CACHE = {}


def get_nc():
    if "nc" not in _NC_CACHE:
        _NC_CACHE["nc"] = build_nc()
    return _NC_CACHE["nc"]


def kernel(x, cos, sin, wq, wk, wv, wo, _trace=False):
    from concourse.bass_utils import run_bass_kernel_spmd

    x = np.asarray(x, dtype=np.float32)
    cos = np.asarray(cos, dtype=np.float32)
    sin = np.asarray(sin, dtype=np.float32)
    wq = np.asarray(wq, dtype=np.float32)
    wk = np.asarray(wk, dtype=np.float32)
    wv = np.asarray(wv, dtype=np.float32)
    wo = np.asarray(wo, dtype=np.float32)

    nc = get_nc()
    in_maps = shard_inputs(x, cos, sin, wq, wk, wv, wo)
    res = run_bass_kernel_spmd(nc, in_maps, list(range(N_CORES)), trace=_trace)
    parts = [
        np.asarray(res.results[c]["out"], dtype=np.float32) for c in range(N_CORES)
    ]
    y = np.stack(
        [sum(parts[b * N_KV + g] for g in range(N_KV)) for b in range(B)], axis=0
    )
    if _trace:
        kernel.last_result = res
    return y


# revision 14
# speedup vs baseline: 1.1924x; 1.0067x over previous
"""Trainium2 Bass kernel for GQA attention with RoPE (B=2, S=1024, HID=2048,
16 q heads / 4 kv heads, head dim 128, causal).

Sharding: 8 cores = 2 batches x 4 kv-head groups. Core c = b*4 + g handles
batch b and kv head g (query heads 4g..4g+3). Each core computes a partial
output y_part = attn_heads @ wo_shard; the host sums the 4 partials per batch.

All tensors fp16 on the wire and in SBUF (host casts inputs; host upcasts and
sums the fp16 partials). Matmuls fp16 (1 cyc/row), except the softmax
denominator which runs as fp8e4m3 DoubleRow over chunk pairs (expst8 is a DVE
cast of the fp16 expst). Causal masking is multiplicative-zero on expst via
gpsimd affine_select (Pool engine), so the DVE stays out of the mask path.

Per-core dataflow:
  Phase A (per 128-row chunk g, software-pipelined):
    x chunk --PE transpose--> xT --mm--> q,k,v (natural); batched RoPE on DVE
    (broadcast cos/sin over the 5 q/k blocks); PE transpose q_rope/k_rope ->
    persistent qT[d,h,s], kT[d,s]; v natural -> vv[s,d].
  Attention (per 256-col tile t, head h, 2-stage pipelined; light doses
  interleave into phase A, the bulk runs after):
    scoresT[sk,sq] = kT_chunk.T @ qT ; exp on ACT -> expst f16 ; diagonal
    causal zeroing on Pool ; den = DoubleRow fp8 ones.T @ expst8 ; U^T
    accumulated fp16 ; rec = 1/den (DVE) ; uT = U^T * rec (DVE, f16).
    wo: y[g,:] = sum_h uT_h.T @ wo_h -> y_sb f16 -> DRAM (one DMA per row).
"""

import sys

import numpy as np

for _p in ("/opt/trn_rl_repo", "/root/.axon_site/_ro/trn_rl_repo"):
    if _p not in sys.path:
        sys.path.append(_p)

from contextlib import ExitStack

import concourse.bass as bass
import concourse.mybir as mybir
from concourse import bacc
from concourse.masks import make_identity
from concourse.tile import TileContext

P = 128           # partitions / head dim / seq chunk
S = 1024          # sequence length
HID = 2048        # model dim
NH = 4            # query heads per core
D = 128           # head dim
TQ = 256          # query macro-tile
NT = S // TQ      # 4 macro tiles
KC = HID // P     # 16 contraction chunks
NSK = S // P      # 8 key chunks
NG = S // P       # 8 row chunks
H2 = D // 2
F32 = mybir.dt.float32
F16 = mybir.dt.float16
F8 = mybir.dt.float8e4
SCALE = 1.0 / float(np.sqrt(D))
AL = mybir.AluOpType
AF = mybir.ActivationFunctionType
DR = mybir.MatmulPerfMode.DoubleRow

USE_F8_DEN = False

N_CORES = 8
B = 2
N_KV = 4


def build_nc():
    nc = bacc.Bacc("TRN2", target_bir_lowering=False, debug=False)
    x_d = nc.declare_dram_parameter("x", [S, HID], F16, isOutput=False)
    cos_d = nc.declare_dram_parameter("cos", [S, D], F16, isOutput=False)
    sin_d = nc.declare_dram_parameter("sin", [S, D], F16, isOutput=False)
    wq_d = nc.declare_dram_parameter("wq", [HID, NH * D], F16, isOutput=False)
    wk_d = nc.declare_dram_parameter("wk", [HID, D], F16, isOutput=False)
    wv_d = nc.declare_dram_parameter("wv", [HID, D], F16, isOutput=False)
    wo_d = nc.declare_dram_parameter("wo", [NH * D, HID], F16, isOutput=False)
    out_d = nc.declare_dram_parameter("out", [S, HID], F16, isOutput=True)

    with TileContext(nc) as tc, ExitStack() as ctx:
        consts = ctx.enter_context(tc.tile_pool(name="consts", bufs=1))
        wpool = ctx.enter_context(tc.tile_pool(name="wpool", bufs=1))
        persist = ctx.enter_context(tc.tile_pool(name="persist", bufs=1))

        # ---- tile declarations (DMAs can start before consts are built) ----
        ident_f32 = consts.tile([P, P], F32, tag="ident_f32")
        ident = consts.tile([P, P], F16, tag="ident")
        ones8 = consts.tile([P, 2, P], F8, tag="ones8")
        ones16 = consts.tile([P, P], F16, tag="ones16")

        # ---- weights (partition-chunked layouts) ----
        wq_sb = wpool.tile([P, KC, NH * D], F16, tag="wq")
        wq_r = wq_d[:].rearrange("(c p) n -> p c n", p=P)
        wkv_sb = wpool.tile([P, KC, 2 * D], F16, tag="wkv")
        wo_sb = wpool.tile([P, NH, HID], F16, tag="wo")
        wo_r = wo_d[:].rearrange("(h p) n -> p h n", p=P)
        cos_sb = wpool.tile([P, NG, D], F16, tag="cos")
        sin_sb = wpool.tile([P, NG, D], F16, tag="sin")

        # persistent transposed activations
        qT_all = persist.tile([P, NH, S], F16, tag="qT")   # [d, h, sq]
        kT = persist.tile([P, S], F16, tag="kT")           # [d, sk]
        vv = persist.tile([P, NSK, D], F16, tag="vv")      # v natural [sk, d]

        # ---- SBUF working pools ----
        pa = ctx.enter_context(tc.tile_pool(name="pa", bufs=2))
        pb = ctx.enter_context(tc.tile_pool(name="pb", bufs=2))
        # ---- PSUM (8 banks): phase A uses qkv 2 + tp 2 (inner stack,
        # released before wo); s 2 + ud 2 persist; y 2 allocated after ----
        ps_s = ctx.enter_context(tc.tile_pool(name="ps_s", bufs=2, space="PSUM"))
        ps_ud = ctx.enter_context(tc.tile_pool(name="ps_ud", bufs=2, space="PSUM"))
        phase_a_ctx = ExitStack()
        ps_qkv = phase_a_ctx.enter_context(
            tc.tile_pool(name="ps_qkv", bufs=1, space="PSUM")
        )
        ps_tp = phase_a_ctx.enter_context(
            tc.tile_pool(name="ps_tp", bufs=2, space="PSUM")
        )

        x_tiles = [None] * NG

        def emit_xdma(g):
            x_nat = pa.tile([P, HID], F16, tag="xnat", bufs=4)
            nc.sync.dma_start(out=x_nat, in_=x_d[g * P : (g + 1) * P, :])
            x_tiles[g] = x_nat

        # x chunks stream on the sync (SP) DGE; weights go on the Pool DGE
        # so the two input streams run in parallel
        emit_xdma(0)
        nc.gpsimd.dma_start(out=wq_sb[:, 0:4, :], in_=wq_r[:, 0:4, :])
        emit_xdma(1)
        nc.gpsimd.dma_start(out=wq_sb[:, 4:8, :], in_=wq_r[:, 4:8, :])

        # ---- constants (gpsimd/DVE work overlapping the DMAs) ----
        make_identity(nc, ident_f32)
        nc.vector.tensor_copy(ident, ident_f32)
        nc.vector.memset(ones8, 1.0)
        nc.vector.memset(ones16, 1.0)

        # warm up the PE clock while the first DMAs are in flight
        warm_ps = ps_s.tile([P, 2 * TQ], F32, tag="s", name="warm")
        for _ in range(30):
            nc.tensor.matmul(warm_ps[:, 0:P], ident, ident, start=True, stop=True)
        warm_drain = pa.tile([P, 4], F32, tag="warmdrain", bufs=1)
        nc.vector.tensor_copy(warm_drain, warm_ps[:, 0:4])

        emit_xdma(2)
        nc.gpsimd.dma_start(out=wq_sb[:, 8:12, :], in_=wq_r[:, 8:12, :])
        nc.gpsimd.dma_start(out=wq_sb[:, 12:16, :], in_=wq_r[:, 12:16, :])
        emit_xdma(3)
        nc.gpsimd.dma_start(
            out=wkv_sb[:, :, 0:D], in_=wk_d[:].rearrange("(c p) n -> p c n", p=P)
        )
        nc.gpsimd.dma_start(
            out=wkv_sb[:, :, D : 2 * D],
            in_=wv_d[:].rearrange("(c p) n -> p c n", p=P),
        )
        nc.gpsimd.dma_start(
            out=cos_sb, in_=cos_d[:].rearrange("(c p) d -> p c d", p=P)
        )
        nc.gpsimd.dma_start(
            out=sin_sb, in_=sin_d[:].rearrange("(c p) d -> p c d", p=P)
        )
        wo_next = [0]

        def emit_wo_dma():
            h = wo_next[0]
            if h < NH:
                nc.gpsimd.dma_start(out=wo_sb[:, h, :], in_=wo_r[:, h, :])
                wo_next[0] += 1

        def transposes(g):
            """x chunk -> xT (PE transpose, f16)."""
            x_nat = x_tiles[g]
            xT = pa.tile([P, KC, P], F16, tag="xT", bufs=2)
            xT_flat = xT.rearrange("p c d -> p (c d)")
            for kb in range(KC // 4):
                tp_ps = ps_tp.tile([P, 5 * P], F16, tag="tp", name="tp")
                for j in range(4):
                    k = 4 * kb + j
                    nc.tensor.transpose(
                        tp_ps[:, j * P : (j + 1) * P],
                        x_nat[:, k * P : (k + 1) * P],
                        ident,
                    )
                if kb % 2 == 0:
                    nc.vector.tensor_copy(
                        xT_flat[:, kb * 4 * P : (kb + 1) * 4 * P], tp_ps[:, 0 : 4 * P]
                    )
                else:
                    nc.scalar.activation(
                        out=xT_flat[:, kb * 4 * P : (kb + 1) * 4 * P],
                        in_=tp_ps[:, 0 : 4 * P],
                        func=AF.Copy,
                    )
            return xT

        def proj(g, xT):
            """q, k, v projections for chunk g (PE, accumulating in PSUM)."""
            qkv_ps = ps_qkv.tile([P, NH * D + 2 * D], F32, tag="qkv")
            q_ps = qkv_ps[:, 0 : NH * D]
            kv_ps = qkv_ps[:, NH * D : NH * D + 2 * D]
            for k in range(KC):
                nc.tensor.matmul(
                    q_ps, xT[:, k, :], wq_sb[:, k, :],
                    start=(k == 0), stop=(k == KC - 1),
                )
            for k in range(KC):
                nc.tensor.matmul(
                    kv_ps, xT[:, k, :], wkv_sb[:, k, :],
                    start=(k == 0), stop=(k == KC - 1),
                )
            # copy-out split across ACT (q) and DVE (kv) to free the bank fast
            qkv_sb = pa.tile([P, NH * D + 2 * D], F16, tag="qkvsb")
            nc.scalar.activation(
                out=qkv_sb[:, 0 : NH * D], in_=q_ps, func=AF.Copy
            )
            nc.vector.tensor_copy(qkv_sb[:, NH * D :], kv_ps)
            return qkv_sb

        def rope_stage(g, qkv_sb):
            """Batched RoPE over the 5 q/k blocks (DVE, broadcast cos/sin)."""
            qk = qkv_sb[:, 0 : 5 * D].rearrange("p (f d) -> p f d", d=D)
            sin_lo = sin_sb[:, g : g + 1, 0:H2].to_broadcast((P, 5, H2))
            sin_hi = sin_sb[:, g : g + 1, H2:D].to_broadcast((P, 5, H2))
            cos_bc = cos_sb[:, g : g + 1, :].to_broadcast((P, 5, D))
            tmp = pa.tile([P, 5, D], F16, tag="ropetmp")
            dst = pa.tile([P, 5, D], F16, tag="qkrope")
            nc.vector.scalar_tensor_tensor(
                out=tmp[:, :, 0:H2], in0=qk[:, :, H2:D], scalar=-1.0,
                in1=sin_lo, op0=AL.mult, op1=AL.mult,
            )
            nc.vector.tensor_tensor(
                out=tmp[:, :, H2:D], in0=qk[:, :, 0:H2], in1=sin_hi, op=AL.mult
            )
            nc.vector.tensor_tensor(out=dst, in0=qk, in1=cos_bc, op=AL.mult)
            nc.vector.tensor_tensor(
                out=dst.rearrange("p f d -> p (f d)"),
                in0=dst.rearrange("p f d -> p (f d)"),
                in1=tmp.rearrange("p f d -> p (f d)"),
                op=AL.add,
            )
            # v copy-out (cast f16)
            nc.vector.tensor_copy(vv[:, g, :], qkv_sb[:, 5 * D : 6 * D])
            return dst

        def rope_transpose(g, dst):
            """Transpose RoPE'd q/k into persistent qT_all / kT."""
            tq_ps = ps_tp.tile([P, 5 * P], F16, tag="tp", name="tq")
            for f in range(5):
                nc.tensor.transpose(
                    tq_ps[:, f * P : (f + 1) * P], dst[:, f, :], ident
                )
            nc.vector.tensor_copy(
                qT_all[:, :, g * P : (g + 1) * P],
                tq_ps[:, 0 : 4 * P].rearrange("p (h d) -> p h d", h=NH),
            )
            nc.scalar.activation(
                out=kT[:, g * P : (g + 1) * P], in_=tq_ps[:, 4 * P : 5 * P],
                func=AF.Copy,
            )

        ropes = [None] * NG
        pend = [None] * NG

        def emit_phase_a(g):
            if g >= 2:
                gg = g - 2
                with nc.named_scope(f"rope_{gg}"):
                    ropes[gg] = rope_stage(gg, pend[gg][1])
            if g < NG:
                if g + 4 < NG:
                    emit_xdma(g + 4)
                if g in (2, 3, 4, 5):
                    emit_wo_dma()
                with nc.named_scope(f"tp_{g}"):
                    xT = transposes(g)
                pend[g] = [xT, None]
            if g >= 1 and g - 1 < NG:
                gg = g - 1
                with nc.named_scope(f"proj_{gg}"):
                    qkv_sb = proj(gg, pend[gg][0])
                pend[gg][1] = qkv_sb
            if g >= 2:
                gg = g - 2
                with nc.named_scope(f"ropeT_{gg}"):
                    rope_transpose(gg, ropes[gg])
                pend[gg] = None

        # ---------- attention ----------
        def scores_head(t, h):
            """scoresT + exp + causal zeroing -> (expst f16, expst8 f8)."""
            qT_h = qT_all[:, h, t * TQ : (t + 1) * TQ]
            expst = pb.tile([P, NSK, TQ], F16, tag="expst", bufs=3)
            expst_flat = expst.rearrange("p c f -> p (c f)")
            if USE_F8_DEN:
                expst8 = pb.tile([P, NSK, TQ], F8, tag="expst8", bufs=3)
            else:
                expst8 = None
            for pi in range(t + 1):
                s_ps = ps_s.tile([P, 2 * TQ], F32, tag="s", name="s")
                for half in range(2):
                    ik = 2 * pi + half
                    nc.tensor.matmul(
                        s_ps[:, half * TQ : (half + 1) * TQ],
                        kT[:, ik * P : (ik + 1) * P], qT_h,
                        start=True, stop=True,
                    )
                nc.scalar.activation(
                    out=expst_flat[:, pi * 2 * TQ : (pi + 1) * 2 * TQ],
                    in_=s_ps, func=AF.Exp, scale=SCALE,
                )
                if pi == t:
                    # causal zeroing of the two diagonal chunks (Pool engine)
                    nc.gpsimd.affine_select(
                        out=expst[:, 2 * t, :], in_=expst[:, 2 * t, :],
                        compare_op=AL.is_ge, fill=0.0,
                        base=0, pattern=[[1, TQ]], channel_multiplier=-1,
                    )
                    nc.gpsimd.affine_select(
                        out=expst[:, 2 * t + 1, :], in_=expst[:, 2 * t + 1, :],
                        compare_op=AL.is_ge, fill=0.0,
                        base=-P, pattern=[[1, TQ]], channel_multiplier=-1,
                    )
                if USE_F8_DEN:
                    # scale by 1+2^-5 so the cast's truncation toward zero
                    # is centered (e4m3 step = 2^-3 relative); den then
                    # averages out instead of sitting ~3% low
                    nc.vector.tensor_scalar_mul(
                        out=expst8.rearrange("p c f -> p (c f)")[
                            :, pi * 2 * TQ : (pi + 1) * 2 * TQ
                        ],
                        in0=expst_flat[:, pi * 2 * TQ : (pi + 1) * 2 * TQ],
                        scalar1=1.03125,
                    )
            return expst, expst8

        def dnpv_head(t, h, exps, uT_t):
            """denominator + PV matmuls, then normalize into uT_t (DVE)."""
            expst, expst8 = exps
            nsk = 2 * (t + 1)
            ud_ps = ps_ud.tile([P, 2 * TQ], F32, tag="ud", name="ud")
            u_ps = ud_ps[:, 0:TQ]
            den_ps = ud_ps[:, TQ : 2 * TQ]
            if USE_F8_DEN:
                for pi in range(t + 1):
                    nc.tensor.matmul(
                        den_ps, ones8,
                        expst8[:, 2 * pi : 2 * pi + 2, :],
                        start=(pi == 0), stop=(pi == t), perf_mode=DR,
                    )
            else:
                for ik in range(nsk):
                    nc.tensor.matmul(
                        den_ps, ones16, expst[:, ik, :],
                        start=(ik == 0), stop=(ik == nsk - 1),
                    )
            rec = pb.tile([P, TQ], F32, tag="rec", bufs=2)
            nc.vector.reciprocal(rec, den_ps)
            for ik in range(nsk):
                nc.tensor.matmul(
                    u_ps, vv[:, ik, :], expst[:, ik, :],
                    start=(ik == 0), stop=(ik == nsk - 1),
                )
            nc.vector.tensor_tensor(
                out=uT_t[:, h, :], in0=u_ps, in1=rec, op=AL.mult
            )

        ps_y_box = [None]

        def wo_stage(t, uT_t):
            for sub in range(2):
                g = 2 * t + sub
                y_sb = pb.tile([P, HID], F16, tag="ysb", bufs=2)
                for n in range(HID // 512):
                    y_ps = ps_y_box[0].tile([P, 512], F32, tag="y", name="y")
                    for h in range(NH):
                        nc.tensor.matmul(
                            y_ps,
                            uT_t[:, h, sub * P : (sub + 1) * P],
                            wo_sb[:, h, n * 512 : (n + 1) * 512],
                            start=(h == 0), stop=(h == NH - 1),
                        )
                    if n % 2 == 0:
                        nc.vector.tensor_copy(
                            y_sb[:, n * 512 : (n + 1) * 512], y_ps
                        )
                    else:
                        nc.scalar.activation(
                            out=y_sb[:, n * 512 : (n + 1) * 512], in_=y_ps,
                            func=AF.Copy,
                        )
                nc.gpsimd.dma_start(
                    out=out_d[g * P : (g + 1) * P, :], in_=y_sb
                )

        steps = [(t, h) for t in range(NT) for h in range(NH)]
        uts = {}
        att_i = [0]
        pending_wo = []

        def emit_attention_step(defer_wo):
            i = att_i[0]
            if i >= len(steps) + 2:
                return False
            if i < len(steps):
                t, h = steps[i]
                if h == 0:
                    uts[t] = pb.tile([P, NH, TQ], F16, tag="uT", name=f"uT{t}", bufs=4)
                with nc.named_scope(f"sc_{t}_{h}"):
                    uts[(t, h)] = scores_head(t, h)
            if 1 <= i < len(steps) + 1:
                t, h = steps[i - 1]
                with nc.named_scope(f"dnpv_{t}_{h}"):
                    dnpv_head(t, h, uts.pop((t, h)), uts[t])
            if i >= 2 and (i - 2) % NH == NH - 1:
                t = steps[i - 2][0]
                if defer_wo:
                    pending_wo.append(t)
                else:
                    while pending_wo:
                        tp = pending_wo.pop(0)
                        with nc.named_scope(f"wo_{tp}"):
                            wo_stage(tp, uts.pop(tp))
                    with nc.named_scope(f"wo_{t}"):
                        wo_stage(t, uts.pop(t))
            att_i[0] += 1
            return True

        def att_ready():
            i = att_i[0]
            if i >= len(steps) + 2:
                return False
            if i < len(steps):
                t, _h = steps[i]
                if 2 * t + 1 > done_g[0]:
                    return False
            return True

        # drive: phase A strictly prioritized; 1 attention step per
        # iteration to fill PE bubbles (wo deferred), the bulk after
        done_g = [-1]
        for g in range(NG + 2):
            emit_phase_a(g)
            done_g[0] = g - 2
            if g >= 3 and att_ready():
                emit_attention_step(defer_wo=True)
        # phase A fully emitted: release its PSUM banks, give wo its own
        phase_a_ctx.close()
        ps_y_box[0] = ctx.enter_context(
            tc.tile_pool(name="ps_y", bufs=2, space="PSUM")
        )
        while pending_wo and att_i[0] > 2:
            tp_ = pending_wo.pop(0)
            with nc.named_scope(f"wo_{tp_}"):
                wo_stage(tp_, uts.pop(tp_))
        while emit_attention_step(defer_wo=False):
            pass

    nc.compile()
    return nc


def shard_inputs(x, cos, sin, wq, wk, wv, wo):
    """Build per-core input maps (fp16): core = b*4 + g."""
    f16 = np.float16
    in_maps = []
    for c in range(N_CORES):
        b, g = divmod(c, N_KV)
        in_maps.append(
            {
                "x": np.ascontiguousarray(x[b], dtype=f16),
                "cos": np.ascontiguousarray(cos, dtype=f16),
                "sin": np.ascontiguousarray(sin, dtype=f16),
                "wq": np.ascontiguousarray(
                    wq[:, g * NH * D : (g + 1) * NH * D], dtype=f16
                ),
                "wk": np.ascontiguousarray(wk[:, g * D : (g + 1) * D], dtype=f16),
                "wv": np.ascontiguousarray(wv[:, g * D : (g + 1) * D], dtype=f16),
                "wo": np.ascontiguousarray(
                    wo[g * NH * D : (g + 1) * NH * D, :], dtype=f16
                ),
            }
        )
    return in_maps


_NC_CACHE = {}


def get_nc():
    if "nc" not in _NC_CACHE:
        _NC_CACHE["nc"] = build_nc()
    return _NC_CACHE["nc"]


def kernel(x, cos, sin, wq, wk, wv, wo, _trace=False):
    from concourse.bass_utils import run_bass_kernel_spmd

    x = np.asarray(x, dtype=np.float32)
    cos = np.asarray(cos, dtype=np.float32)
    sin = np.asarray(sin, dtype=np.float32)
    wq = np.asarray(wq, dtype=np.float32)
    wk = np.asarray(wk, dtype=np.float32)
    wv = np.asarray(wv, dtype=np.float32)
    wo = np.asarray(wo, dtype=np.float32)

    nc = get_nc()
    in_maps = shard_inputs(x, cos, sin, wq, wk, wv, wo)
    res = run_bass_kernel_spmd(nc, in_maps, list(range(N_CORES)), trace=_trace)
    parts = [
        np.asarray(res.results[c]["out"], dtype=np.float32) for c in range(N_CORES)
    ]
    y = np.stack(
        [sum(parts[b * N_KV + g] for g in range(N_KV)) for b in range(B)], axis=0
    )
    if _trace:
        kernel.last_result = res
    return y
